# revision 1
# baseline (speedup 1.0000x reference)
"""GenAttentionMask packed-ragged kernel for 8 Trainium2 NeuronCores.

Semantics (matches the reference):
  for each sample i: take mask[i, :s_i, :s_i], flatten to s_i^2 elements,
  tile it num_heads times; concatenate all pieces -> 1D fp16 buffer of
  length num_heads * sum(s_i^2).

Device strategy (memory-bound, pure data movement):
  - Host packs the ragged blocks into one contiguous stream P (this is the
    sharding/layout step; ~1/17 of total traffic).
  - P is split into 8 equal ranges of Q elements, one per core: perfect
    load balance regardless of the ragged lengths (padding < 128 els/core).
  - Every core runs the SAME static program (SPMD requires one NEFF): for
    each tile of its range: DMA HBM->SBUF once, DMA SBUF->HBM num_heads
    times (optionally as ONE broadcast-AP DMA). Read 1x, write 16x == the
    roofline traffic. Tiles are graduated (small first tile) so the serial
    load prefix before the first store is short; loads ride the sync HWDGE
    ring, broadcast stores the scalar ring, so loads pipeline under stores.
    Measured walls (8 cores active): ~354 GB/s/core pure-read, ~331
    GB/s/core pure broadcast-write vs the ~358 GB/s/core HBM share, so the
    kernel sits within ~4% of the per-invocation hardware floor
    (read-serial + write-wall + ~3.7us For_i drain/barrier).
  - Host assembles the final ragged concat from the per-core outputs with
    contiguous slice copies only (it knows exactly where every tile landed).
"""

import numpy as np

P_DIM = 128
MAX_COLS = 4096  # 1 MiB fp16 tiles
NCORES = 8

_NC_CACHE = {}


FIRST_COLS = 512


def _tile_sizes_cols(cpp):
    """Graduated tile sizes: a small first tile shortens the serial load
    prefix (the pipeline's fill); the rest in MAX_COLS chunks."""
    left = int(cpp)
    sizes = []
    if left > 2 * FIRST_COLS:
        sizes.append(FIRST_COLS)
        left -= FIRST_COLS
    while left > 0:
        c = min(MAX_COLS, left)
        sizes.append(c)
        left -= c
    return sizes


def _build_nc(sizes_cols, R, reps=1, broadcast=True, alt=False, bufs=4,
              nsplit=1, loop_n=0):
    """One static SPMD program: per tile, one load then R stores (or one
    broadcast store writing all R replicas; nsplit>1 splits the replica dim
    across that many broadcast DMAs).

    reps>1 unrolls the whole job back-to-back; loop_n>0 instead wraps one
    job in a Tile For_i loop (a full drain per back-edge), which measures
    isolated per-invocation time. Both are benchmarking aids.
    """
    import concourse.bacc as bacc
    import concourse.mybir as mybir
    from concourse import tile

    sizes_cols = list(sizes_cols)
    Q = P_DIM * sum(sizes_cols)
    nc = bacc.Bacc("TRN2", target_bir_lowering=False, debug=False,
                   num_devices=NCORES)
    inp = nc.dram_tensor("inp", [Q], mybir.dt.float16,
                         kind="ExternalInput").ap()
    out = nc.dram_tensor("out", [R * Q], mybir.dt.float16,
                         kind="ExternalOutput").ap()

    def body_phased(pool):
        # All DMAs on one HWDGE ring, loads emitted first: ring FIFO gives a
        # pure-read phase then a pure-write phase (no HBM R/W interleaving).
        tiles = []
        off = 0
        for c in sizes_cols:
            els = P_DIM * c
            t = pool.tile([P_DIM, c], mybir.dt.float16)
            nc.scalar.dma_start(
                t[:], inp[off:off + els].rearrange("(p c) -> p c", p=P_DIM))
            tiles.append((t, off, c, els))
            off += els
        rr = R // nsplit
        for t, off, c, els in tiles:
            for v in range(nsplit):
                r0 = v * rr
                dst = out[R * off + r0 * els:
                          R * off + (r0 + rr) * els].rearrange(
                              "(r p c) -> p r c", r=rr, p=P_DIM)
                nc.scalar.dma_start(
                    dst, t[:].unsqueeze(1).broadcast_to([P_DIM, rr, c]))

    def body(pool):
        off = 0
        for j, c in enumerate(sizes_cols):
            els = P_DIM * c
            t = pool.tile([P_DIM, c], mybir.dt.float16)
            nc.sync.dma_start(
                t[:], inp[off:off + els].rearrange("(p c) -> p c", p=P_DIM))
            if broadcast:
                rr = R // nsplit
                for v in range(nsplit):
                    r0 = v * rr
                    dst = out[R * off + r0 * els:
                              R * off + (r0 + rr) * els].rearrange(
                                  "(r p c) -> p r c", r=rr, p=P_DIM)
                    src = t[:].unsqueeze(1).broadcast_to([P_DIM, rr, c])
                    e = nc.scalar if (not alt or (j + v) % 2 == 0) \
                        else nc.sync
                    e.dma_start(dst, src)
            else:
                for r in range(R):
                    dst = out[R * off + r * els:
                              R * off + (r + 1) * els].rearrange(
                                  "(p c) -> p c", p=P_DIM)
                    e = nc.scalar if r % 2 else nc.sync
                    e.dma_start(dst, t[:])
            off += els

    b = body_phased if alt == "phase" else body
    with tile.TileContext(nc) as tc:
        with tc.tile_pool(name="sbuf", bufs=bufs) as pool:
            if loop_n:
                with tc.For_i(0, loop_n, 1):
                    b(pool)
            else:
                for _ in range(reps):
                    b(pool)
    nc.compile()
    return nc


def _get_nc(sizes_cols, R, reps=1, broadcast=True, alt=False, bufs=4,
            nsplit=1, loop_n=0):
    key = (tuple(sizes_cols), R, reps, broadcast, alt, bufs, nsplit, loop_n)
    if key not in _NC_CACHE:
        _NC_CACHE[key] = _build_nc(sizes_cols, R, reps, broadcast, alt,
                                   bufs, nsplit, loop_n)
    return _NC_CACHE[key]


def _prod_nc(sizes_cols, R, loop_n=0):
    """The production kernel variant: per tile, a load on the sync ring
    and ONE broadcast store (all R replicas) on the scalar ring, bufs=6.
    Identical DMA sequence to _raw_loop_nc's body."""
    return _get_nc(sizes_cols, R, 1, True, False, 6, 1, loop_n=loop_n)


def _raw_loop_nc(sizes_cols, R, loop_n):
    """Timing variant: the same body as _prod_nc, wrapped in a raw
    two-engine loop synchronized purely by semaphores instead of Tile's
    For_i (whose back-edge costs ~2.5us: two all-engine barriers + a Pool
    sem reset). Isolation is preserved: at iteration k the SP engine
    top-waits semS >= nd*k, i.e. every prior store's completion receipt
    (which also makes tile reuse WAR-safe), and each store waits its own
    load via semL. Sem counts stay monotonic (no reset), so loop_n must
    keep nd*loop_n below the semaphore counter range; the caller bounds
    it."""
    import concourse.bacc as bacc
    import concourse.mybir as mybir
    from contextlib import ExitStack

    sizes_cols = list(sizes_cols)
    Q = P_DIM * sum(sizes_cols)
    nd = len(sizes_cols) * 16
    nc = bacc.Bacc("TRN2", target_bir_lowering=False, debug=False,
                   num_devices=NCORES)
    inp = nc.dram_tensor("inp", [Q], mybir.dt.float16,
                         kind="ExternalInput").ap()
    out = nc.dram_tensor("out", [R * Q], mybir.dt.float16,
                         kind="ExternalOutput").ap()
    semL = nc.alloc_semaphore("semL")
    semS = nc.alloc_semaphore("semS")
    engines = [mybir.EngineType.SP, mybir.EngineType.Activation]

    with ExitStack() as st:
        tiles = [
            st.enter_context(
                nc.sbuf_tensor(f"t{j}", [P_DIM, c], mybir.dt.float16))
            for j, c in enumerate(sizes_cols)
        ]
        nc.sync.sem_clear(semL)
        nc.sync.sem_clear(semS)
        nc.all_engine_barrier()
        with nc.Fori(0, nd * loop_n, nd, engines=engines) as i:
            nc.sync.wait_ge(semS, i)
            off = 0
            for j, c in enumerate(sizes_cols):
                els = P_DIM * c
                nc.sync.dma_start(
                    tiles[j][:],
                    inp[off:off + els].rearrange("(p c) -> p c", p=P_DIM)
                ).then_inc(semL, 16)
                off += els
            off = 0
            for j, c in enumerate(sizes_cols):
                els = P_DIM * c
                nc.scalar.wait_ge(semL, i + 16 * (j + 1))
                dst = out[R * off:R * off + R * els].rearrange(
                    "(r p c) -> p r c", r=R, p=P_DIM)
                nc.scalar.dma_start(
                    dst,
                    tiles[j][:].unsqueeze(1).broadcast_to([P_DIM, R, c])
                ).then_inc(semS, 16)
                off += els
        nc.sync.wait_ge(semS, nd * loop_n)
        nc.all_engine_barrier()
    nc.compile()
    return nc


def _plan(lens):
    """Pack layout: per-sample packed sizes/offsets and per-core quota."""
    s2 = lens.astype(np.int64) ** 2
    T = int(s2.sum())
    pbase = np.zeros(len(lens) + 1, np.int64)
    pbase[1:] = np.cumsum(s2)
    cpp = -(-T // (NCORES * P_DIM))  # ceil cols-per-partition per core
    sizes_cols = _tile_sizes_cols(cpp)
    Q = P_DIM * cpp
    # element offset of each tile within a core's range (+ sentinel Q)
    tprefix = np.zeros(len(sizes_cols) + 1, np.int64)
    tprefix[1:] = np.cumsum([P_DIM * c for c in sizes_cols])
    return s2, T, pbase, Q, sizes_cols, tprefix


def _pack_stream(am, lens, T, pbase, Q):
    Pstream = np.zeros(NCORES * Q, dtype=np.float16)
    for i in range(len(lens)):
        s = int(lens[i])
        Pstream[pbase[i]:pbase[i + 1]].reshape(s, s)[...] = am[i, :s, :s]
    return Pstream


def _assemble(outs, lens, s2, pbase, Q, tprefix, R):
    """outs[k] flat fp16 of length R*Q; returns the final packed concat.

    Device layout: replica r of the tile at [tprefix[j], tprefix[j+1]) lives
    at out[R*tprefix[j] + r*els_j + (g - tprefix[j])].
    """
    T = int(pbase[-1])
    F = np.empty(R * T, dtype=np.float16)
    # global cut positions: every core boundary + tile boundary
    tile_cuts = np.concatenate(
        [k * Q + tprefix[:-1] for k in range(NCORES)] + [[NCORES * Q]])
    for i in range(len(lens)):
        sz = int(s2[i])
        g0 = int(pbase[i])
        g1 = g0 + sz
        # cuts strictly inside (g0, g1)
        inner = tile_cuts[(tile_cuts > g0) & (tile_cuts < g1)]
        cuts = [g0] + [int(x) for x in inner] + [g1]
        for h in range(R):
            dst0 = R * g0 + h * sz
            for a, b in zip(cuts[:-1], cuts[1:]):
                k = a // Q
                loc = a - k * Q
                j = int(np.searchsorted(tprefix, loc, side="right")) - 1
                els_j = int(tprefix[j + 1] - tprefix[j])
                st = R * int(tprefix[j]) + h * els_j + (loc - int(tprefix[j]))
                F[dst0 + (a - g0):dst0 + (b - g0)] = outs[k][st:st + (b - a)]
    return F


def kernel(attention_mask, seq_lengths, num_heads):
    am = np.asarray(attention_mask)
    if am.dtype != np.float16:
        am = am.astype(np.float16)
    lens = np.asarray(seq_lengths).astype(np.int64)
    R = int(np.asarray(num_heads))

    s2, T, pbase, Q, sizes_cols, tprefix = _plan(lens)
    if R == 0 or T == 0:
        return np.zeros(R * T, dtype=np.float16)
    Pstream = _pack_stream(am, lens, T, pbase, Q)
    in_maps = [{"inp": Pstream[k * Q:(k + 1) * Q]} for k in range(NCORES)]

    try:
        outs = _run_device(sizes_cols, R, in_maps)
    except Exception:
        # Transient device loss (NRT_EXEC_UNIT_UNRECOVERABLE): the terminal
        # self-recovers after a pause, but only a FRESH process can
        # reconnect — the in-process jax client stays wedged. Retry in
        # subprocesses.
        outs = _run_device_subprocess(in_maps, sizes_cols, R)
    return _assemble(outs, lens, s2, pbase, Q, tprefix, R)


def _run_device(sizes_cols, R, in_maps):
    from concourse.bass_utils import run_bass_kernel_spmd
    nc = _prod_nc(sizes_cols, R)
    res = run_bass_kernel_spmd(nc, in_maps, core_ids=list(range(NCORES)))
    return [np.asarray(res.results[k]["out"]).reshape(-1)
            for k in range(NCORES)]


def _subproc_entry(tmpdir):
    """Runs inside the retry subprocess: load staged inputs, run, save."""
    import os
    meta = np.load(os.path.join(tmpdir, "meta.npy"))
    R, ntiles = int(meta[0]), int(meta[1])
    sizes_cols = [int(x) for x in meta[2:2 + ntiles]]
    Pstream = np.load(os.path.join(tmpdir, "pstream.npy"))
    Q = Pstream.size // NCORES
    in_maps = [{"inp": Pstream[k * Q:(k + 1) * Q]} for k in range(NCORES)]
    outs = _run_device(sizes_cols, R, in_maps)
    np.save(os.path.join(tmpdir, "outs.npy"), np.stack(outs))


def _run_device_subprocess(in_maps, sizes_cols, R, attempts=3):
    import os
    import subprocess
    import sys
    import tempfile
    import time

    kdir = os.path.dirname(os.path.abspath(__file__))
    with tempfile.TemporaryDirectory() as td:
        meta = np.array([R, len(sizes_cols)] + list(sizes_cols),
                        np.int64)
        np.save(os.path.join(td, "meta.npy"), meta)
        Pstream = np.concatenate([m["inp"] for m in in_maps])
        np.save(os.path.join(td, "pstream.npy"), Pstream)
        code = (f"import sys; sys.path.insert(0, {kdir!r}); "
                f"import kernel; kernel._subproc_entry({td!r})")
        err = None
        for i in range(attempts):
            time.sleep(90 if i else 10)  # let the terminal recover first
            p = subprocess.run([sys.executable, "-c", code],
                               capture_output=True, text=True,
                               timeout=1800)
            if p.returncode == 0 and os.path.exists(
                    os.path.join(td, "outs.npy")):
                stacked = np.load(os.path.join(td, "outs.npy"))
                return [stacked[k] for k in range(NCORES)]
            err = p.stderr[-2000:]
        raise RuntimeError(f"device retries exhausted: {err}")



# revision 2
# speedup vs baseline: 1.0412x; 1.0412x over previous
"""GenAttentionMask packed-ragged kernel for 8 Trainium2 NeuronCores.

Semantics (matches the reference):
  for each sample i: take mask[i, :s_i, :s_i], flatten to s_i^2 elements,
  tile it num_heads times; concatenate all pieces -> 1D fp16 buffer of
  length num_heads * sum(s_i^2).

Device strategy (memory-bound, pure data movement):
  - Host packs the ragged blocks into one contiguous stream P, split into
    8 equal ranges of Q = 128*cpp elements (perfect load balance).
  - Per core the whole job is ONE SBUF tile [128, cpp] (rows of cpp*2
    bytes). HBM-write throughput on this part scales with the DMA
    descriptor run length (bytes contiguous in both src row and dst):
    measured 328 GB/s at 17KB runs, ~340 at 34KB, ~353 at 51KB, ~357 at
    65KB (descriptor length field caps runs at 65535B). A run can only
    be as long as the contiguous data in one SBUF partition row, so the
    kernel DUPLICATES each row K=3x with DVE tensor_copies (~750 GB/s,
    off the DMA ports) into a wide tile [128, 3*cpp], then stores
    replicas in G=R//K groups of K with 3*cpp*2-byte runs (51KB for the
    target shapes, the longest that fits the 65535B descriptor cap).
  - Single-shot schedule, all DMAs on one HWDGE ring (ACT) so FIFO order
    replaces semaphores: load -> remainder replica store (runs while the
    DVE builds the wide tile) -> G group-stores. Only the group store
    waits on a semaphore (DVE completion).
  - Host assembles the final ragged concat from the per-core outputs
    with numpy reshape/transpose + contiguous slice copies only.
"""

import numpy as np

P_DIM = 128
NCORES = 8
DESC_CAP = 65535  # max bytes per DMA descriptor run

_NC_CACHE = {}


def _dup_factor(cpp, R):
    """Replicas per wide SBUF row: bounded by the descriptor length cap
    (runs of K*cpp*2 bytes) and by R itself."""
    return max(1, min(R, DESC_CAP // (2 * cpp)))


def _build_prod(cpp, R, loop_n=0):
    """The production NEFF. loop_n>0 wraps the body in a raw Fori loop
    with semaphore-isolated iterations (for timing): iteration k+1's
    load waits on all of iteration k's store receipts."""
    import concourse.bacc as bacc
    import concourse.mybir as mybir
    from contextlib import ExitStack

    K = _dup_factor(cpp, R)
    G = R // K
    REM = R - G * K
    Q = P_DIM * cpp
    c3 = K * cpp
    nstores = (1 if G else 0) + (1 if REM else 0)

    nc = bacc.Bacc("TRN2", target_bir_lowering=False, debug=False,
                   num_devices=NCORES)
    inp = nc.dram_tensor("inp", [Q], mybir.dt.float16,
                         kind="ExternalInput").ap()
    out = nc.dram_tensor("out", [R * Q], mybir.dt.float16,
                         kind="ExternalOutput").ap()
    semL = nc.alloc_semaphore("semL")
    semC = nc.alloc_semaphore("semC")
    semS = nc.alloc_semaphore("semS")
    ACT = mybir.EngineType.Activation
    DVE = mybir.EngineType.DVE

    with ExitStack() as st:
        w = st.enter_context(
            nc.sbuf_tensor("w", [P_DIM, c3], mybir.dt.float16))

        def body(i):
            # i: iteration index (RuntimeValue) or None for single-shot
            ld = nc.scalar.dma_start(
                w[:, 0:cpp],
                inp[0:Q].rearrange("(p c) -> p c", p=P_DIM))
            ld.then_inc(semL, 16)
            if REM:
                # remainder replicas from the base columns; FIFO after the
                # load on the same ring, overlaps the DVE build
                dst = out[G * K * Q:R * Q].rearrange(
                    "(r p c) -> p r c", r=REM, p=P_DIM)
                nc.scalar.dma_start(
                    dst,
                    w[:, 0:cpp].unsqueeze(1).broadcast_to([P_DIM, REM, cpp])
                ).then_inc(semS, 16)
            if K > 1:
                nc.vector.wait_ge(
                    semL, 16 if i is None else 16 * i + 16)
                for j in range(1, K):
                    cp = nc.vector.tensor_copy(
                        w[:, j * cpp:(j + 1) * cpp], w[:, 0:cpp])
                cp.then_inc(semC, 1)
                nc.scalar.wait_ge(semC, 1 if i is None else i + 1)
            if G:
                dst = out[0:G * K * Q].rearrange(
                    "(g p c) -> p g c", g=G, p=P_DIM)
                nc.scalar.dma_start(
                    dst, w[:].unsqueeze(1).broadcast_to([P_DIM, G, c3])
                ).then_inc(semS, 16)

        nc.scalar.sem_clear(semL)
        nc.scalar.sem_clear(semC)
        nc.scalar.sem_clear(semS)
        nc.all_engine_barrier()
        if loop_n:
            engines = [ACT, DVE] if K > 1 else [ACT]
            with nc.Fori(0, loop_n, 1, engines=engines) as i:
                nc.scalar.wait_ge(semS, 16 * nstores * i)
                body(i)
            nc.scalar.wait_ge(semS, 16 * nstores * loop_n)
        else:
            body(None)
            nc.scalar.wait_ge(semS, 16 * nstores)
        nc.all_engine_barrier()
    nc.compile()
    return nc


def _get_nc(cpp, R, loop_n=0):
    key = (cpp, R, loop_n)
    if key not in _NC_CACHE:
        _NC_CACHE[key] = _build_prod(cpp, R, loop_n)
    return _NC_CACHE[key]


def _prod_nc(sizes_cols, R, loop_n=0):
    return _get_nc(sum(sizes_cols), R, loop_n)


def _raw_loop_nc(sizes_cols, R, loop_n):
    """Timing variant: the production body in a raw semaphore-isolated
    loop (see _build_prod)."""
    return _get_nc(sum(sizes_cols), R, loop_n)


def _plan(lens):
    """Pack layout: per-sample packed sizes/offsets and per-core quota."""
    s2 = lens.astype(np.int64) ** 2
    T = int(s2.sum())
    pbase = np.zeros(len(lens) + 1, np.int64)
    pbase[1:] = np.cumsum(s2)
    cpp = -(-T // (NCORES * P_DIM))  # ceil cols-per-partition per core
    sizes_cols = [int(cpp)]
    Q = P_DIM * cpp
    tprefix = np.array([0, Q], np.int64)
    return s2, T, pbase, Q, sizes_cols, tprefix


def _pack_stream(am, lens, T, pbase, Q):
    Pstream = np.zeros(NCORES * Q, dtype=np.float16)
    for i in range(len(lens)):
        s = int(lens[i])
        Pstream[pbase[i]:pbase[i + 1]].reshape(s, s)[...] = am[i, :s, :s]
    return Pstream


def _reorder_device_out(dev, cpp, R):
    """Device out buffer -> [R, Q] replica-major view matching the packed
    stream layout. Device layout: G groups of K replicas (one wide-row
    store each: group g at [g*K*Q, (g+1)*K*Q), partition-major rows of
    K*cpp), then REM replicas partition-major."""
    K = _dup_factor(cpp, R)
    G = R // K
    REM = R - G * K
    Q = P_DIM * cpp
    parts = []
    if G:
        main = dev[:G * K * Q].reshape(G, P_DIM, K, cpp)
        parts.append(main.transpose(0, 2, 1, 3).reshape(G * K, Q))
    if REM:
        parts.append(dev[G * K * Q:R * Q].reshape(REM, Q))
    return np.concatenate(parts, axis=0) if len(parts) > 1 else parts[0]


def _assemble(outs, lens, s2, pbase, Q, tprefix, R):
    """outs[k]: [R, Q] replica-major per-core buffers; returns the final
    packed concat."""
    T = int(pbase[-1])
    F = np.empty(R * T, dtype=np.float16)
    core_cuts = np.array([k * Q for k in range(NCORES + 1)], np.int64)
    for i in range(len(lens)):
        sz = int(s2[i])
        g0 = int(pbase[i])
        g1 = g0 + sz
        inner = core_cuts[(core_cuts > g0) & (core_cuts < g1)]
        cuts = [g0] + [int(x) for x in inner] + [g1]
        for h in range(R):
            dst0 = R * g0 + h * sz
            for a, b in zip(cuts[:-1], cuts[1:]):
                k = a // Q
                loc = a - k * Q
                F[dst0 + (a - g0):dst0 + (b - g0)] = \
                    outs[k][h, loc:loc + (b - a)]
    return F


def kernel(attention_mask, seq_lengths, num_heads):
    am = np.asarray(attention_mask)
    if am.dtype != np.float16:
        am = am.astype(np.float16)
    lens = np.asarray(seq_lengths).astype(np.int64)
    R = int(np.asarray(num_heads))

    s2, T, pbase, Q, sizes_cols, tprefix = _plan(lens)
    if R == 0 or T == 0:
        return np.zeros(R * T, dtype=np.float16)
    Pstream = _pack_stream(am, lens, T, pbase, Q)
    in_maps = [{"inp": Pstream[k * Q:(k + 1) * Q]} for k in range(NCORES)]

    try:
        outs = _run_device(sizes_cols, R, in_maps)
    except Exception:
        # Transient device loss (NRT_EXEC_UNIT_UNRECOVERABLE): the terminal
        # self-recovers after a pause, but only a FRESH process can
        # reconnect — the in-process jax client stays wedged. Retry in
        # subprocesses.
        outs = _run_device_subprocess(in_maps, sizes_cols, R)
    cpp = sizes_cols[0]
    reps = [_reorder_device_out(d, cpp, R) for d in outs]
    return _assemble(reps, lens, s2, pbase, Q, tprefix, R)


def _run_device(sizes_cols, R, in_maps):
    from concourse.bass_utils import run_bass_kernel_spmd
    nc = _prod_nc(sizes_cols, R)
    res = run_bass_kernel_spmd(nc, in_maps, core_ids=list(range(NCORES)))
    return [np.asarray(res.results[k]["out"]).reshape(-1)
            for k in range(NCORES)]


def _subproc_entry(tmpdir):
    """Runs inside the retry subprocess: load staged inputs, run, save."""
    import os
    meta = np.load(os.path.join(tmpdir, "meta.npy"))
    R, ntiles = int(meta[0]), int(meta[1])
    sizes_cols = [int(x) for x in meta[2:2 + ntiles]]
    Pstream = np.load(os.path.join(tmpdir, "pstream.npy"))
    Q = Pstream.size // NCORES
    in_maps = [{"inp": Pstream[k * Q:(k + 1) * Q]} for k in range(NCORES)]
    outs = _run_device(sizes_cols, R, in_maps)
    np.save(os.path.join(tmpdir, "outs.npy"), np.stack(outs))


def _run_device_subprocess(in_maps, sizes_cols, R, attempts=3):
    import os
    import subprocess
    import sys
    import tempfile
    import time

    kdir = os.path.dirname(os.path.abspath(__file__))
    with tempfile.TemporaryDirectory() as td:
        meta = np.array([R, len(sizes_cols)] + list(sizes_cols),
                        np.int64)
        np.save(os.path.join(td, "meta.npy"), meta)
        Pstream = np.concatenate([m["inp"] for m in in_maps])
        np.save(os.path.join(td, "pstream.npy"), Pstream)
        code = (f"import sys; sys.path.insert(0, {kdir!r}); "
                f"import kernel; kernel._subproc_entry({td!r})")
        err = None
        for i in range(attempts):
            time.sleep(90 if i else 10)  # let the terminal recover first
            p = subprocess.run([sys.executable, "-c", code],
                               capture_output=True, text=True,
                               timeout=1800)
            if p.returncode == 0 and os.path.exists(
                    os.path.join(td, "outs.npy")):
                stacked = np.load(os.path.join(td, "outs.npy"))
                return [stacked[k] for k in range(NCORES)]
            err = p.stderr[-2000:]
        raise RuntimeError(f"device retries exhausted: {err}")


# revision 4
# speedup vs baseline: 1.0437x; 1.0024x over previous
"""GenAttentionMask packed-ragged kernel for 8 Trainium2 NeuronCores.

Semantics (matches the reference):
  for each sample i: take mask[i, :s_i, :s_i], flatten to s_i^2 elements,
  tile it num_heads times; concatenate all pieces -> 1D fp16 buffer of
  length num_heads * sum(s_i^2).

Device strategy (memory-bound, pure data movement):
  - Host packs the ragged blocks into one contiguous stream P, split into
    8 equal ranges of Q = 128*cpp elements (perfect load balance).
  - Per core the whole job is ONE SBUF tile [128, cpp] (rows of cpp*2
    bytes). HBM-write throughput on this part scales with the DMA
    descriptor run length (bytes contiguous in both src row and dst):
    measured 328 GB/s at 17KB runs, ~340 at 34KB, ~353 at 51KB, ~357 at
    65KB (descriptor length field caps runs at 65535B). A run can only
    be as long as the contiguous data in one SBUF partition row, so the
    kernel DUPLICATES each row K=3x with DVE tensor_copies (~750 GB/s,
    off the DMA ports) into a wide tile [128, 3*cpp], then stores
    replicas in G=R//K groups of K with 3*cpp*2-byte runs (51KB for the
    target shapes, the longest that fits the 65535B descriptor cap).
  - Single-shot schedule, all DMAs on one HWDGE ring (ACT) so FIFO order
    replaces semaphores: load -> remainder replica store (runs while the
    DVE builds the wide tile) -> G group-stores. Only the group store
    waits on a semaphore (DVE completion).
  - Host assembles the final ragged concat from the per-core outputs
    with numpy reshape/transpose + contiguous slice copies only.
"""

import numpy as np

P_DIM = 128
NCORES = 8
DESC_CAP = 65535  # max bytes per DMA descriptor run

_NC_CACHE = {}


def _dup_factor(cpp, R):
    """Replicas per wide SBUF row: bounded by the descriptor length cap
    (runs of K*cpp*2 bytes) and by R itself."""
    return max(1, min(R, DESC_CAP // (2 * cpp)))


def _build_prod(cpp, R, loop_n=0):
    """The production NEFF. loop_n>0 wraps the body in a raw Fori loop
    with semaphore-isolated iterations (for timing): iteration k+1's
    load waits on all of iteration k's store receipts."""
    import concourse.bacc as bacc
    import concourse.mybir as mybir
    from contextlib import ExitStack

    K = _dup_factor(cpp, R)
    G = R // K
    REM = R - G * K
    Q = P_DIM * cpp
    c3 = K * cpp
    cpad = -(-c3 // 2048) * 2048  # 4KB-aligned run starts (~1.4% on BW)
    nstores = (1 if G else 0) + (1 if REM else 0)

    nc = bacc.Bacc("TRN2", target_bir_lowering=False, debug=False,
                   num_devices=NCORES)
    inp = nc.dram_tensor("inp", [Q], mybir.dt.float16,
                         kind="ExternalInput").ap()
    out = nc.dram_tensor("out", [G * P_DIM * cpad + REM * Q],
                         mybir.dt.float16, kind="ExternalOutput").ap()
    semL = nc.alloc_semaphore("semL")
    semC = nc.alloc_semaphore("semC")
    semS = nc.alloc_semaphore("semS")
    ACT = mybir.EngineType.Activation
    DVE = mybir.EngineType.DVE

    with ExitStack() as st:
        w = st.enter_context(
            nc.sbuf_tensor("w", [P_DIM, c3], mybir.dt.float16))

        def body(i):
            # i: iteration index (RuntimeValue) or None for single-shot
            ld = nc.scalar.dma_start(
                w[:, 0:cpp],
                inp[0:Q].rearrange("(p c) -> p c", p=P_DIM))
            ld.then_inc(semL, 16)
            if REM:
                # remainder replicas from the base columns; FIFO after the
                # load on the same ring, overlaps the DVE build
                off0 = G * P_DIM * cpad
                dst = out[off0:off0 + REM * Q].rearrange(
                    "(r p c) -> p r c", r=REM, p=P_DIM)
                nc.scalar.dma_start(
                    dst,
                    w[:, 0:cpp].unsqueeze(1).broadcast_to([P_DIM, REM, cpp])
                ).then_inc(semS, 16)
            if K > 1:
                nc.vector.wait_ge(
                    semL, 16 if i is None else 16 * i + 16)
                for j in range(1, K):
                    cp = nc.vector.tensor_copy(
                        w[:, j * cpp:(j + 1) * cpp], w[:, 0:cpp])
                cp.then_inc(semC, 1)
                nc.scalar.wait_ge(semC, 1 if i is None else i + 1)
            if G:
                dst = out[0:G * P_DIM * cpad].rearrange(
                    "(g p c) -> p g c", g=G, p=P_DIM)[:, :, 0:c3]
                nc.scalar.dma_start(
                    dst, w[:].unsqueeze(1).broadcast_to([P_DIM, G, c3])
                ).then_inc(semS, 16)

        nc.scalar.sem_clear(semL)
        nc.scalar.sem_clear(semC)
        nc.scalar.sem_clear(semS)
        nc.all_engine_barrier()
        if loop_n:
            engines = [ACT, DVE] if K > 1 else [ACT]
            with nc.Fori(0, loop_n, 1, engines=engines) as i:
                nc.scalar.wait_ge(semS, 16 * nstores * i)
                body(i)
            nc.scalar.wait_ge(semS, 16 * nstores * loop_n)
        else:
            body(None)
            nc.scalar.wait_ge(semS, 16 * nstores)
        nc.all_engine_barrier()
    nc.compile()
    return nc


def _get_nc(cpp, R, loop_n=0):
    key = (cpp, R, loop_n)
    if key not in _NC_CACHE:
        _NC_CACHE[key] = _build_prod(cpp, R, loop_n)
    return _NC_CACHE[key]


def _prod_nc(sizes_cols, R, loop_n=0):
    return _get_nc(sum(sizes_cols), R, loop_n)


def _raw_loop_nc(sizes_cols, R, loop_n):
    """Timing variant: the production body in a raw semaphore-isolated
    loop (see _build_prod)."""
    return _get_nc(sum(sizes_cols), R, loop_n)


def _plan(lens):
    """Pack layout: per-sample packed sizes/offsets and per-core quota."""
    s2 = lens.astype(np.int64) ** 2
    T = int(s2.sum())
    pbase = np.zeros(len(lens) + 1, np.int64)
    pbase[1:] = np.cumsum(s2)
    cpp = -(-T // (NCORES * P_DIM))  # ceil cols-per-partition per core
    sizes_cols = [int(cpp)]
    Q = P_DIM * cpp
    tprefix = np.array([0, Q], np.int64)
    return s2, T, pbase, Q, sizes_cols, tprefix


def _pack_stream(am, lens, T, pbase, Q):
    Pstream = np.zeros(NCORES * Q, dtype=np.float16)
    for i in range(len(lens)):
        s = int(lens[i])
        Pstream[pbase[i]:pbase[i + 1]].reshape(s, s)[...] = am[i, :s, :s]
    return Pstream


def _reorder_device_out(dev, cpp, R):
    """Device out buffer -> [R, Q] replica-major view matching the packed
    stream layout. Device layout: G groups of K replicas (one wide-row
    store each: group g at [g*K*Q, (g+1)*K*Q), partition-major rows of
    K*cpp), then REM replicas partition-major."""
    K = _dup_factor(cpp, R)
    G = R // K
    REM = R - G * K
    Q = P_DIM * cpp
    c3 = K * cpp
    cpad = -(-c3 // 2048) * 2048
    parts = []
    if G:
        main = dev[:G * P_DIM * cpad].reshape(G, P_DIM, cpad)[:, :, 0:c3]
        main = main.reshape(G, P_DIM, K, cpp)
        parts.append(main.transpose(0, 2, 1, 3).reshape(G * K, Q))
    if REM:
        off0 = G * P_DIM * cpad
        parts.append(dev[off0:off0 + REM * Q].reshape(REM, Q))
    return np.concatenate(parts, axis=0) if len(parts) > 1 else parts[0]


def _assemble(outs, lens, s2, pbase, Q, tprefix, R):
    """outs[k]: [R, Q] replica-major per-core buffers; returns the final
    packed concat."""
    T = int(pbase[-1])
    F = np.empty(R * T, dtype=np.float16)
    core_cuts = np.array([k * Q for k in range(NCORES + 1)], np.int64)
    for i in range(len(lens)):
        sz = int(s2[i])
        g0 = int(pbase[i])
        g1 = g0 + sz
        inner = core_cuts[(core_cuts > g0) & (core_cuts < g1)]
        cuts = [g0] + [int(x) for x in inner] + [g1]
        for h in range(R):
            dst0 = R * g0 + h * sz
            for a, b in zip(cuts[:-1], cuts[1:]):
                k = a // Q
                loc = a - k * Q
                F[dst0 + (a - g0):dst0 + (b - g0)] = \
                    outs[k][h, loc:loc + (b - a)]
    return F


def kernel(attention_mask, seq_lengths, num_heads):
    am = np.asarray(attention_mask)
    if am.dtype != np.float16:
        am = am.astype(np.float16)
    lens = np.asarray(seq_lengths).astype(np.int64)
    R = int(np.asarray(num_heads))

    s2, T, pbase, Q, sizes_cols, tprefix = _plan(lens)
    if R == 0 or T == 0:
        return np.zeros(R * T, dtype=np.float16)
    Pstream = _pack_stream(am, lens, T, pbase, Q)
    in_maps = [{"inp": Pstream[k * Q:(k + 1) * Q]} for k in range(NCORES)]

    try:
        outs = _run_device(sizes_cols, R, in_maps)
    except Exception:
        # Transient device loss (NRT_EXEC_UNIT_UNRECOVERABLE): the terminal
        # self-recovers after a pause, but only a FRESH process can
        # reconnect — the in-process jax client stays wedged. Retry in
        # subprocesses.
        outs = _run_device_subprocess(in_maps, sizes_cols, R)
    cpp = sizes_cols[0]
    reps = [_reorder_device_out(d, cpp, R) for d in outs]
    return _assemble(reps, lens, s2, pbase, Q, tprefix, R)


def _run_device(sizes_cols, R, in_maps):
    from concourse.bass_utils import run_bass_kernel_spmd
    nc = _prod_nc(sizes_cols, R)
    res = run_bass_kernel_spmd(nc, in_maps, core_ids=list(range(NCORES)))
    return [np.asarray(res.results[k]["out"]).reshape(-1)
            for k in range(NCORES)]


def _subproc_entry(tmpdir):
    """Runs inside the retry subprocess: load staged inputs, run, save."""
    import os
    meta = np.load(os.path.join(tmpdir, "meta.npy"))
    R, ntiles = int(meta[0]), int(meta[1])
    sizes_cols = [int(x) for x in meta[2:2 + ntiles]]
    Pstream = np.load(os.path.join(tmpdir, "pstream.npy"))
    Q = Pstream.size // NCORES
    in_maps = [{"inp": Pstream[k * Q:(k + 1) * Q]} for k in range(NCORES)]
    outs = _run_device(sizes_cols, R, in_maps)
    np.save(os.path.join(tmpdir, "outs.npy"), np.stack(outs))


def _run_device_subprocess(in_maps, sizes_cols, R, attempts=3):
    import os
    import subprocess
    import sys
    import tempfile
    import time

    kdir = os.path.dirname(os.path.abspath(__file__))
    with tempfile.TemporaryDirectory() as td:
        meta = np.array([R, len(sizes_cols)] + list(sizes_cols),
                        np.int64)
        np.save(os.path.join(td, "meta.npy"), meta)
        Pstream = np.concatenate([m["inp"] for m in in_maps])
        np.save(os.path.join(td, "pstream.npy"), Pstream)
        code = (f"import sys; sys.path.insert(0, {kdir!r}); "
                f"import kernel; kernel._subproc_entry({td!r})")
        err = None
        for i in range(attempts):
            time.sleep(90 if i else 10)  # let the terminal recover first
            p = subprocess.run([sys.executable, "-c", code],
                               capture_output=True, text=True,
                               timeout=1800)
            if p.returncode == 0 and os.path.exists(
                    os.path.join(td, "outs.npy")):
                stacked = np.load(os.path.join(td, "outs.npy"))
                return [stacked[k] for k in range(NCORES)]
            err = p.stderr[-2000:]
        raise RuntimeError(f"device retries exhausted: {err}")


# revision 5
# speedup vs baseline: 1.0585x; 1.0142x over previous
"""GenAttentionMask packed-ragged kernel for 8 Trainium2 NeuronCores.

Semantics (matches the reference):
  for each sample i: take mask[i, :s_i, :s_i], flatten to s_i^2 elements,
  tile it num_heads times; concatenate all pieces -> 1D fp16 buffer of
  length num_heads * sum(s_i^2).

Device strategy (memory-bound, pure data movement):
  - Host packs the ragged blocks into one contiguous stream P, split into
    8 equal ranges of Q = 128*cpp elements (perfect load balance).
  - Per core the whole job is ONE SBUF tile [128, cpp] (rows of cpp*2
    bytes). HBM-write throughput on this part scales with the DMA
    descriptor run length (bytes contiguous in both src row and dst):
    measured 328 GB/s at 17KB runs, ~340 at 34KB, ~353 at 51KB, ~357 at
    65KB (descriptor length field caps runs at 65535B). A run can only
    be as long as the contiguous data in one SBUF partition row, so the
    kernel DUPLICATES each row K=3x with DVE tensor_copies (~750 GB/s,
    off the DMA ports) into a wide tile [128, 3*cpp], then stores
    replicas in G=R//K groups of K with 3*cpp*2-byte runs (51KB for the
    target shapes, the longest that fits the 65535B descriptor cap).
  - Single-shot schedule, all DMAs on one HWDGE ring (ACT) so FIFO order
    replaces semaphores: load -> remainder replica store (runs while the
    DVE builds the wide tile) -> G group-stores. Only the group store
    waits on a semaphore (DVE completion).
  - Host assembles the final ragged concat from the per-core outputs
    with numpy reshape/transpose + contiguous slice copies only.
"""

import numpy as np

P_DIM = 128
NCORES = 8
DESC_CAP = 65535  # max bytes per DMA descriptor run

_NC_CACHE = {}


def _dup_factor(cpp, R):
    """Replicas per wide SBUF row: bounded by the descriptor length cap
    (runs of K*cpp*2 bytes) and by R itself."""
    return max(1, min(R, DESC_CAP // (2 * cpp)))


def _sem_rate(cpp, R):
    """semS increment per timed-loop iteration (16 per store)."""
    K = _dup_factor(cpp, R)
    G = R // K
    REM = R - G * K
    return 16 * (G + (1 if REM else 0))


def _build_prod(cpp, R, loop_n=0):
    """The production NEFF. loop_n>0 wraps the body in a raw Fori loop
    with semaphore-isolated iterations (for timing): iteration k+1's
    load waits on all of iteration k's store receipts."""
    import concourse.bacc as bacc
    import concourse.mybir as mybir
    from contextlib import ExitStack

    K = _dup_factor(cpp, R)
    G = R // K
    REM = R - G * K
    Q = P_DIM * cpp
    c3 = K * cpp
    cpad = -(-c3 // 2048) * 2048  # 4KB-aligned run starts (~1.4% on BW)
    nstores = G + (1 if REM else 0)

    nc = bacc.Bacc("TRN2", target_bir_lowering=False, debug=False,
                   num_devices=NCORES)
    inp = nc.dram_tensor("inp", [Q], mybir.dt.float16,
                         kind="ExternalInput").ap()
    out = nc.dram_tensor("out", [G * P_DIM * cpad + REM * Q],
                         mybir.dt.float16, kind="ExternalOutput").ap()
    semL = nc.alloc_semaphore("semL")
    semC = nc.alloc_semaphore("semC")
    semS = nc.alloc_semaphore("semS")
    ACT = mybir.EngineType.Activation
    DVE = mybir.EngineType.DVE

    with ExitStack() as st:
        w = st.enter_context(
            nc.sbuf_tensor("w", [P_DIM, c3], mybir.dt.float16))

        def body(i):
            # i: iteration index (RuntimeValue) or None for single-shot
            ld = nc.scalar.dma_start(
                w[:, 0:cpp],
                inp[0:Q].rearrange("(p c) -> p c", p=P_DIM))
            ld.then_inc(semL, 16)
            if REM:
                # remainder replicas from the base columns; FIFO after the
                # load on the same ring, overlaps the DVE build
                off0 = G * P_DIM * cpad
                dst = out[off0:off0 + REM * Q].rearrange(
                    "(r p c) -> p r c", r=REM, p=P_DIM)
                nc.scalar.dma_start(
                    dst,
                    w[:, 0:cpp].unsqueeze(1).broadcast_to([P_DIM, REM, cpp])
                ).then_inc(semS, 16)
            if K > 1:
                nc.vector.wait_ge(
                    semL, 16 if i is None else 16 * i + 16)
                for j in range(1, K):
                    cp = nc.vector.tensor_copy(
                        w[:, j * cpp:(j + 1) * cpp], w[:, 0:cpp])
                cp.then_inc(semC, 1)
                nc.scalar.wait_ge(semC, 1 if i is None else i + 1)
            for g in range(G):
                # G separate PLAIN stores of the wide tile to 4KB-aligned
                # regions: measured ~352 GB/s vs ~345 for one
                # broadcast-AP store over all groups
                base = g * P_DIM * cpad
                dst = out[base:base + P_DIM * cpad].rearrange(
                    "(p c) -> p c", p=P_DIM)[:, 0:c3]
                nc.scalar.dma_start(dst, w[:]).then_inc(semS, 16)

        nc.scalar.sem_clear(semL)
        nc.scalar.sem_clear(semC)
        nc.scalar.sem_clear(semS)
        nc.all_engine_barrier()
        if loop_n:
            engines = [ACT, DVE] if K > 1 else [ACT]
            with nc.Fori(0, loop_n, 1, engines=engines) as i:
                nc.scalar.wait_ge(semS, 16 * nstores * i)
                body(i)
            nc.scalar.wait_ge(semS, 16 * nstores * loop_n)
        else:
            body(None)
            nc.scalar.wait_ge(semS, 16 * nstores)
        nc.all_engine_barrier()
    nc.compile()
    return nc


def _get_nc(cpp, R, loop_n=0):
    key = (cpp, R, loop_n)
    if key not in _NC_CACHE:
        _NC_CACHE[key] = _build_prod(cpp, R, loop_n)
    return _NC_CACHE[key]


def _prod_nc(sizes_cols, R, loop_n=0):
    return _get_nc(sum(sizes_cols), R, loop_n)


def _raw_loop_nc(sizes_cols, R, loop_n):
    """Timing variant: the production body in a raw semaphore-isolated
    loop (see _build_prod)."""
    return _get_nc(sum(sizes_cols), R, loop_n)


def _plan(lens):
    """Pack layout: per-sample packed sizes/offsets and per-core quota."""
    s2 = lens.astype(np.int64) ** 2
    T = int(s2.sum())
    pbase = np.zeros(len(lens) + 1, np.int64)
    pbase[1:] = np.cumsum(s2)
    cpp = -(-T // (NCORES * P_DIM))  # ceil cols-per-partition per core
    sizes_cols = [int(cpp)]
    Q = P_DIM * cpp
    tprefix = np.array([0, Q], np.int64)
    return s2, T, pbase, Q, sizes_cols, tprefix


def _pack_stream(am, lens, T, pbase, Q):
    Pstream = np.zeros(NCORES * Q, dtype=np.float16)
    for i in range(len(lens)):
        s = int(lens[i])
        Pstream[pbase[i]:pbase[i + 1]].reshape(s, s)[...] = am[i, :s, :s]
    return Pstream


def _reorder_device_out(dev, cpp, R):
    """Device out buffer -> [R, Q] replica-major view matching the packed
    stream layout. Device layout: G groups of K replicas (one wide-row
    store each: group g at [g*K*Q, (g+1)*K*Q), partition-major rows of
    K*cpp), then REM replicas partition-major."""
    K = _dup_factor(cpp, R)
    G = R // K
    REM = R - G * K
    Q = P_DIM * cpp
    c3 = K * cpp
    cpad = -(-c3 // 2048) * 2048
    parts = []
    if G:
        main = dev[:G * P_DIM * cpad].reshape(G, P_DIM, cpad)[:, :, 0:c3]
        main = main.reshape(G, P_DIM, K, cpp)
        parts.append(main.transpose(0, 2, 1, 3).reshape(G * K, Q))
    if REM:
        off0 = G * P_DIM * cpad
        parts.append(dev[off0:off0 + REM * Q].reshape(REM, Q))
    return np.concatenate(parts, axis=0) if len(parts) > 1 else parts[0]


def _assemble(outs, lens, s2, pbase, Q, tprefix, R):
    """outs[k]: [R, Q] replica-major per-core buffers; returns the final
    packed concat."""
    T = int(pbase[-1])
    F = np.empty(R * T, dtype=np.float16)
    core_cuts = np.array([k * Q for k in range(NCORES + 1)], np.int64)
    for i in range(len(lens)):
        sz = int(s2[i])
        g0 = int(pbase[i])
        g1 = g0 + sz
        inner = core_cuts[(core_cuts > g0) & (core_cuts < g1)]
        cuts = [g0] + [int(x) for x in inner] + [g1]
        for h in range(R):
            dst0 = R * g0 + h * sz
            for a, b in zip(cuts[:-1], cuts[1:]):
                k = a // Q
                loc = a - k * Q
                F[dst0 + (a - g0):dst0 + (b - g0)] = \
                    outs[k][h, loc:loc + (b - a)]
    return F


def kernel(attention_mask, seq_lengths, num_heads):
    am = np.asarray(attention_mask)
    if am.dtype != np.float16:
        am = am.astype(np.float16)
    lens = np.asarray(seq_lengths).astype(np.int64)
    R = int(np.asarray(num_heads))

    s2, T, pbase, Q, sizes_cols, tprefix = _plan(lens)
    if R == 0 or T == 0:
        return np.zeros(R * T, dtype=np.float16)
    Pstream = _pack_stream(am, lens, T, pbase, Q)
    in_maps = [{"inp": Pstream[k * Q:(k + 1) * Q]} for k in range(NCORES)]

    try:
        outs = _run_device(sizes_cols, R, in_maps)
    except Exception:
        # Transient device loss (NRT_EXEC_UNIT_UNRECOVERABLE): the terminal
        # self-recovers after a pause, but only a FRESH process can
        # reconnect — the in-process jax client stays wedged. Retry in
        # subprocesses.
        outs = _run_device_subprocess(in_maps, sizes_cols, R)
    cpp = sizes_cols[0]
    reps = [_reorder_device_out(d, cpp, R) for d in outs]
    return _assemble(reps, lens, s2, pbase, Q, tprefix, R)


def _run_device(sizes_cols, R, in_maps):
    from concourse.bass_utils import run_bass_kernel_spmd
    nc = _prod_nc(sizes_cols, R)
    res = run_bass_kernel_spmd(nc, in_maps, core_ids=list(range(NCORES)))
    return [np.asarray(res.results[k]["out"]).reshape(-1)
            for k in range(NCORES)]


def _subproc_entry(tmpdir):
    """Runs inside the retry subprocess: load staged inputs, run, save."""
    import os
    meta = np.load(os.path.join(tmpdir, "meta.npy"))
    R, ntiles = int(meta[0]), int(meta[1])
    sizes_cols = [int(x) for x in meta[2:2 + ntiles]]
    Pstream = np.load(os.path.join(tmpdir, "pstream.npy"))
    Q = Pstream.size // NCORES
    in_maps = [{"inp": Pstream[k * Q:(k + 1) * Q]} for k in range(NCORES)]
    outs = _run_device(sizes_cols, R, in_maps)
    np.save(os.path.join(tmpdir, "outs.npy"), np.stack(outs))


def _run_device_subprocess(in_maps, sizes_cols, R, attempts=3):
    import os
    import subprocess
    import sys
    import tempfile
    import time

    kdir = os.path.dirname(os.path.abspath(__file__))
    with tempfile.TemporaryDirectory() as td:
        meta = np.array([R, len(sizes_cols)] + list(sizes_cols),
                        np.int64)
        np.save(os.path.join(td, "meta.npy"), meta)
        Pstream = np.concatenate([m["inp"] for m in in_maps])
        np.save(os.path.join(td, "pstream.npy"), Pstream)
        code = (f"import sys; sys.path.insert(0, {kdir!r}); "
                f"import kernel; kernel._subproc_entry({td!r})")
        err = None
        for i in range(attempts):
            time.sleep(90 if i else 10)  # let the terminal recover first
            p = subprocess.run([sys.executable, "-c", code],
                               capture_output=True, text=True,
                               timeout=1800)
            if p.returncode == 0 and os.path.exists(
                    os.path.join(td, "outs.npy")):
                stacked = np.load(os.path.join(td, "outs.npy"))
                return [stacked[k] for k in range(NCORES)]
            err = p.stderr[-2000:]
        raise RuntimeError(f"device retries exhausted: {err}")
